# revision 6
# baseline (speedup 1.0000x reference)
"""DeepKoopman linear-decoder kernel for 8 TRN2 NeuronCores.

Data-parallel over batch: each core gets B/8 = 256 rows. All weights
replicated. Inside each core:

  - Encoder MLP (32 -> 512 -> 512 -> 512 -> 128, relu between) runs in
    feature-major layout: activations are stored [features, rows] so every
    layer is out = W.T @ actT with weights stationary and no transposes
    between layers. Rows enter/leave via PE-transposes at DRAM boundaries.
  - The M=64-step linear recurrence z' = z@A + u@B runs as two accumulating
    matmuls per step into one PSUM bank (A on z, block-diagonal B on the
    pre-transposed u), interleaved between encoder tiles so the serial
    chain hides under encoder compute.
  - Decoder x = z@C + C_b is one matmul per step off the scan state.

Matmul dtype is float32r (TF32-like fp32 path at full PE rate; operands are
rounded during the PSUM->SBUF copies that produce them).
"""
import sys
sys.path.insert(0, "/opt/trn_rl_repo")

import numpy as np
from contextlib import ExitStack

import concourse.bacc as bacc
import concourse.tile as tile
from concourse.tile import TileContext
from concourse import mybir
from concourse.bass_utils import run_bass_kernel_spmd
from concourse.masks import make_identity

dt = mybir.dt
AF = mybir.ActivationFunctionType

B, M, SD, CD, LD, HW = 2048, 64, 32, 8, 128, 512
NCORES = 8
BL = B // NCORES          # 256 batch rows per core
RT = BL * M               # 16384 encoder rows per core
P = 128
NT = RT // 512            # 32 encoder tiles of 512 rows

_prog_cache = {}


def _build(mm_dt):
    """Build the per-core program (SPMD: same program, 8 input slices)."""
    nc = bacc.Bacc()

    x_k_d = nc.declare_dram_parameter("x_k", [BL, SD], dt.float32, isOutput=False)
    u_d = nc.declare_dram_parameter("u", [BL, M * CD], dt.float32, isOutput=False)
    xn_d = nc.declare_dram_parameter("x_next", [RT, SD], dt.float32, isOutput=False)
    w0_d = nc.declare_dram_parameter("w0", [SD, HW], dt.float32, isOutput=False)
    b0_d = nc.declare_dram_parameter("b0", [HW], dt.float32, isOutput=False)
    w1_d = nc.declare_dram_parameter("w1", [HW, HW], dt.float32, isOutput=False)
    b1_d = nc.declare_dram_parameter("b1", [HW], dt.float32, isOutput=False)
    w2_d = nc.declare_dram_parameter("w2", [HW, HW], dt.float32, isOutput=False)
    b2_d = nc.declare_dram_parameter("b2", [HW], dt.float32, isOutput=False)
    w3_d = nc.declare_dram_parameter("w3", [HW, LD], dt.float32, isOutput=False)
    b3_d = nc.declare_dram_parameter("b3", [LD], dt.float32, isOutput=False)
    aw_d = nc.declare_dram_parameter("aw", [LD, LD], dt.float32, isOutput=False)
    bw_d = nc.declare_dram_parameter("bw", [CD, LD], dt.float32, isOutput=False)
    cw_d = nc.declare_dram_parameter("cw", [LD, SD], dt.float32, isOutput=False)
    cb_d = nc.declare_dram_parameter("cb", [SD], dt.float32, isOutput=False)

    zp_d = nc.declare_dram_parameter("z_pred", [BL, M, LD], dt.float32, isOutput=True)
    xp_d = nc.declare_dram_parameter("x_pred", [BL, M, SD], dt.float32, isOutput=True)
    zt_d = nc.declare_dram_parameter("z_target", [RT, LD], dt.float32, isOutput=True)

    # DRAM views
    xn_v = xn_d[:].rearrange("(l p) c -> p l c", p=P)        # [128, 128blk, 32]
    xk_v = x_k_d[:].rearrange("(l p) c -> p l c", p=P)       # [128, 2, 32]
    u_v = u_d[:].rearrange("(l p) c -> p l c", p=P)          # [128, 2, 512]
    zt_v = zt_d[:].rearrange("(g p) l -> p g l", p=P)        # [128, 128blk, 128]
    zp_v = zp_d[:].rearrange("(ch p) m l -> p ch m l", p=P)  # [128, 2, 64, 128]
    xp_v = xp_d[:].rearrange("(ch p) m c -> p ch m c", p=P)  # [128, 2, 64, 32]

    with TileContext(nc) as tc, ExitStack() as ctx:
        consts = ctx.enter_context(tc.tile_pool(name="consts", bufs=1))
        wst = ctx.enter_context(tc.tile_pool(name="wst", bufs=1))
        xin = ctx.enter_context(tc.tile_pool(name="xin", bufs=2))
        acts = ctx.enter_context(tc.tile_pool(name="acts", bufs=2))
        ztsb = ctx.enter_context(tc.tile_pool(name="ztsb", bufs=2))
        zts = ctx.enter_context(tc.tile_pool(name="zts", bufs=2))
        zpool = ctx.enter_context(tc.tile_pool(name="zpool", bufs=3))
        zps = ctx.enter_context(tc.tile_pool(name="zps", bufs=2))
        xps = ctx.enter_context(tc.tile_pool(name="xps", bufs=1))
        pe_ps = ctx.enter_context(tc.tile_pool(name="pe_ps", bufs=3, space="PSUM"))
        tr_ps = ctx.enter_context(tc.tile_pool(name="tr_ps", bufs=3, space="PSUM"))
        sc_ps = ctx.enter_context(tc.tile_pool(name="sc_ps", bufs=1, space="PSUM"))

        # ---- constants: weights (fp32 load -> mm_dt cast), biases, identities
        ident32 = consts.tile([P, P], dt.float32)
        make_identity(nc, ident32)
        identr = consts.tile([P, P], mm_dt)
        nc.vector.tensor_copy(identr, ident32)

        # W0 [32, 512]
        w0_st = wst.tile([SD, HW], dt.float32, tag="w0stage")
        nc.sync.dma_start(out=w0_st, in_=w0_d[:])
        w0r = consts.tile([SD, HW], mm_dt)
        nc.vector.tensor_copy(w0r, w0_st)
        # W1/W2 [128, 4, 512] (k-chunk on dim1)
        w1r = consts.tile([P, 4, HW], mm_dt)
        w2r = consts.tile([P, 4, HW], mm_dt)
        for wd, wr in ((w1_d, w1r), (w2_d, w2r)):
            st = wst.tile([P, 4, HW], dt.float32, tag="wstage")
            nc.sync.dma_start(out=st, in_=wd[:].rearrange("(k p) o -> p k o", p=P))
            nc.vector.tensor_copy(wr, st)
        # W3 [128, 4, 128]
        w3_st = wst.tile([P, 4, LD], dt.float32, tag="w3stage")
        nc.sync.dma_start(out=w3_st, in_=w3_d[:].rearrange("(k p) o -> p k o", p=P))
        w3r = consts.tile([P, 4, LD], mm_dt)
        nc.vector.tensor_copy(w3r, w3_st)
        # A_w [128, 128]
        aw_st = wst.tile([P, LD], dt.float32, tag="awstage")
        nc.sync.dma_start(out=aw_st, in_=aw_d[:])
        awr = consts.tile([P, LD], mm_dt)
        nc.vector.tensor_copy(awr, aw_st)
        # C_w [128, 32]
        cw_st = wst.tile([P, SD], dt.float32, tag="cwstage")
        nc.sync.dma_start(out=cw_st, in_=cw_d[:])
        cwr = consts.tile([P, SD], mm_dt)
        nc.vector.tensor_copy(cwr, cw_st)
        # B_w -> block-diagonal Bblk [128(16m x 8c), 16, 128]
        bw_st = wst.tile([CD, LD], dt.float32, tag="bwstage")
        nc.sync.dma_start(out=bw_st, in_=bw_d[:])
        bblk_st = wst.tile([P, 16, LD], dt.float32, tag="bblkstage")
        nc.vector.memset(bblk_st, 0.0)
        for mp in range(16):
            nc.sync.dma_start(out=bblk_st[8 * mp:8 * mp + 8, mp, :], in_=bw_st)
        bblk = consts.tile([P, 16, LD], mm_dt)
        nc.vector.tensor_copy(bblk, bblk_st)
        # biases: [128, 4] per hidden layer (col j = chunk j), b3/cb as [*, 1]
        b01 = consts.tile([P, 4], dt.float32, tag="b0t")
        nc.sync.dma_start(out=b01, in_=b0_d[:].rearrange("(k p) -> p k", p=P))
        b11 = consts.tile([P, 4], dt.float32, tag="b1t")
        nc.sync.dma_start(out=b11, in_=b1_d[:].rearrange("(k p) -> p k", p=P))
        b21 = consts.tile([P, 4], dt.float32, tag="b2t")
        nc.sync.dma_start(out=b21, in_=b2_d[:].rearrange("(k p) -> p k", p=P))
        b31 = consts.tile([P, 1], dt.float32, tag="b3t")
        nc.sync.dma_start(out=b31, in_=b3_d[:].rearrange("(p o) -> p o", o=1))
        cb1 = consts.tile([SD, 1], dt.float32, tag="cbt")
        nc.sync.dma_start(out=cb1, in_=cb_d[:].rearrange("(p o) -> p o", o=1))
        bias_tiles = [b01, b11, b21]

        # ---- u: load + transpose into (m,c)-major tiles for the scan
        u_in = wst.tile([P, 2, M * CD], dt.float32, tag="uin")
        nc.sync.dma_start(out=u_in, in_=u_v)
        uT = [consts.tile([P, 2, P], mm_dt, tag=f"uT{j}", name=f"uT{j}") for j in range(4)]
        for ch in range(2):
            for j in range(4):
                pst = tr_ps.tile([P, P], dt.float32, tag="trp")
                nc.tensor.transpose(pst, u_in[:, ch, j * P:(j + 1) * P], ident32)
                nc.vector.tensor_copy(uT[j][:, ch, :], pst)

        # ---- encoder helper (feature-major). Returns nothing; writes outputs
        # via the supplied sinks.
        def encode_tile(xT, n, z_sink):
            """xT: [32, n] mm_dt tile view. z_sink(psum_ap) consumes the L3
            PSUM output [128, n]."""
            h_prev = None
            for li, (wr, nchunk) in enumerate(((w0r, 4), (w1r, 4), (w2r, 4))):
                h = acts.tile([P, 4, 512], mm_dt, tag=f"h{li}")
                for j in range(4):
                    ps = pe_ps.tile([P, 512], dt.float32, tag="ps")
                    if li == 0:
                        nc.tensor.matmul(ps[:, :n], w0r[:, j * P:(j + 1) * P],
                                         xT, start=True, stop=True)
                    else:
                        for kk in range(4):
                            nc.tensor.matmul(
                                ps[:, :n], wr[:, kk, j * P:(j + 1) * P],
                                h_prev[:, kk, :n],
                                start=(kk == 0), stop=(kk == 3))
                    # relu + bias, alternate engines
                    if j % 2 == 0:
                        nc.scalar.activation(h[:, j, :n], ps[:, :n], AF.Relu,
                                             bias=bias_tiles[li][:, j:j + 1],
                                             scale=1.0)
                    else:
                        nc.vector.tensor_scalar(
                            out=h[:, j, :n], in0=ps[:, :n],
                            scalar1=bias_tiles[li][:, j:j + 1], scalar2=0.0,
                            op0=mybir.AluOpType.add, op1=mybir.AluOpType.max)
                h_prev = h
            ps = pe_ps.tile([P, 512], dt.float32, tag="ps")
            for kk in range(4):
                nc.tensor.matmul(ps[:, :n], w3r[:, kk, :], h_prev[:, kk, :n],
                                 start=(kk == 0), stop=(kk == 3))
            z_sink(ps[:, :n])

        # ---- x_k encode -> initial scan state z_cur [128, 256] mm_dt
        xk_in = wst.tile([P, 2, SD], dt.float32, tag="xkin")
        nc.sync.dma_start(out=xk_in, in_=xk_v)
        xkT = acts.tile([SD, 2, P], mm_dt, tag="xT")
        for ch in range(2):
            pst = tr_ps.tile([SD, P], dt.float32, tag="trp")
            nc.tensor.transpose(pst, xk_in[:, ch, :], ident32)
            nc.vector.tensor_copy(xkT[:, ch, :], pst)

        z_state = [None]

        def zk_sink(ps):
            z0 = zpool.tile([P, 2, P], mm_dt, tag="zcur")
            nc.vector.tensor_scalar(out=z0[:, 0, :], in0=ps[:, 0:P],
                                    scalar1=b31, scalar2=None,
                                    op0=mybir.AluOpType.add)
            nc.vector.tensor_scalar(out=z0[:, 1, :], in0=ps[:, P:2 * P],
                                    scalar1=b31, scalar2=None,
                                    op0=mybir.AluOpType.add)
            z_state[0] = z0

        encode_tile(xkT, BL, zk_sink)

        # ---- scan step emission
        scan_ctx = {}

        def scan_step(m):
            q, mi = divmod(m, 16)
            if mi == 0:
                scan_ctx["zq"] = [zps.tile([P, 16, LD], dt.float32, tag=f"zps{c}", name=f"zps{c}")
                                  for c in range(2)]
            psA = sc_ps.tile([P, 2 * P], dt.float32, tag="sps")
            nc.tensor.matmul(psA, awr, z_state[0], start=True, stop=False)
            nc.tensor.matmul(psA, bblk[:, m % 16, :], uT[q], start=False, stop=True)
            znext = zpool.tile([P, 2, P], mm_dt, tag="zcur")
            nc.vector.tensor_copy(znext, psA.rearrange("p (c b) -> p c b", c=2))
            z_state[0] = znext
            # z_pred staging (transpose back to row-major)
            for ch in range(2):
                trp = tr_ps.tile([P, P], mm_dt, tag="trp")
                nc.tensor.transpose(trp, znext[:, ch, :], identr)
                nc.scalar.activation(scan_ctx["zq"][ch][:, mi, :], trp, AF.Copy)
            if mi == 15:
                for ch in range(2):
                    nc.sync.dma_start(out=zp_v[:, ch, q * 16:(q + 1) * 16, :],
                                      in_=scan_ctx["zq"][ch])
            # decoder x_pred
            psC = sc_ps.tile([SD, 2 * P], dt.float32, tag="dps")
            nc.tensor.matmul(psC, cwr, znext.rearrange("p c b -> p (c b)"),
                             start=True, stop=True)
            xP = acts.tile([SD, 2 * P], dt.float32, tag="xP")
            nc.vector.tensor_scalar(out=xP, in0=psC, scalar1=cb1, scalar2=None,
                                    op0=mybir.AluOpType.add)
            for ch in range(2):
                trx = tr_ps.tile([P, SD], dt.float32, tag="trp")
                nc.tensor.transpose(trx, xP[:, ch * P:(ch + 1) * P],
                                    ident32[0:SD, 0:SD])
                nc.vector.tensor_copy(scan_ctx["xp"][ch][:, m, :], trx)

        scan_ctx["xp"] = [xps.tile([P, M, SD], dt.float32, tag=f"xps{c}", name=f"xps{c}")
                          for c in range(2)]

        # ---- main encoder loop over 32 tiles of 512 rows, scan interleaved
        for t in range(NT):
            if t % 8 == 0:
                xt_in = xin.tile([P, 32, SD], dt.float32, tag="xin")
                nc.sync.dma_start(out=xt_in,
                                  in_=xn_v[:, (t // 8) * 32:(t // 8) * 32 + 32, :])
            if t % 2 == 0:
                zt_stage = zts.tile([P, 8, LD], dt.float32, tag="zts")
            xT = acts.tile([SD, 4, P], mm_dt, tag="xT")
            for q in range(4):
                pst = tr_ps.tile([SD, P], dt.float32, tag="trp")
                nc.tensor.transpose(pst, xt_in[:, (t % 8) * 4 + q, :], ident32)
                nc.vector.tensor_copy(xT[:, q, :], pst)

            zt_tile = ztsb.tile([P, 512], dt.float32, tag="ztsb")

            def zt_sink(ps, t=t, zt_tile=zt_tile, zt_stage=zt_stage):
                nc.vector.tensor_scalar(out=zt_tile, in0=ps, scalar1=b31,
                                        scalar2=None, op0=mybir.AluOpType.add)
                for q in range(4):
                    trp = tr_ps.tile([P, P], dt.float32, tag="trp")
                    nc.tensor.transpose(trp, zt_tile[:, q * P:(q + 1) * P], ident32)
                    gg = (t % 2) * 4 + q
                    if q % 2 == 0:
                        nc.scalar.activation(zt_stage[:, gg, :], trp, AF.Copy)
                    else:
                        nc.vector.tensor_copy(zt_stage[:, gg, :], trp)

            encode_tile(xT.rearrange("p q b -> p (q b)"), 512, zt_sink)
            if t % 2 == 1:
                g0 = (t - 1) * 4
                nc.sync.dma_start(out=zt_v[:, g0:g0 + 8, :], in_=zt_stage)
            scan_step(2 * t)
            scan_step(2 * t + 1)

        # ---- final x_pred flush
        for ch in range(2):
            nc.sync.dma_start(out=xp_v[:, ch, :, :], in_=scan_ctx["xp"][ch])

    nc.compile()
    return nc


def _get_prog(mm_dt_name):
    if mm_dt_name not in _prog_cache:
        _prog_cache[mm_dt_name] = _build(getattr(dt, mm_dt_name))
    return _prog_cache[mm_dt_name]


def kernel(x_k, u_seq, x_next_seq,
           enc_w0, enc_b0, enc_w1, enc_b1, enc_w2, enc_b2, enc_w3, enc_b3,
           A_w, B_w, C_w, C_b, mm_dt_name="float32r", trace=False):
    nc = _get_prog(mm_dt_name)
    x_k = np.ascontiguousarray(x_k, dtype=np.float32)
    u_seq = np.ascontiguousarray(u_seq, dtype=np.float32)
    x_next_seq = np.ascontiguousarray(x_next_seq, dtype=np.float32)
    weights = dict(
        w0=np.ascontiguousarray(enc_w0, np.float32),
        b0=np.ascontiguousarray(enc_b0, np.float32),
        w1=np.ascontiguousarray(enc_w1, np.float32),
        b1=np.ascontiguousarray(enc_b1, np.float32),
        w2=np.ascontiguousarray(enc_w2, np.float32),
        b2=np.ascontiguousarray(enc_b2, np.float32),
        w3=np.ascontiguousarray(enc_w3, np.float32),
        b3=np.ascontiguousarray(enc_b3, np.float32),
        aw=np.ascontiguousarray(A_w, np.float32),
        bw=np.ascontiguousarray(B_w, np.float32),
        cw=np.ascontiguousarray(C_w, np.float32),
        cb=np.ascontiguousarray(C_b, np.float32),
    )
    in_maps = []
    for c in range(NCORES):
        sl = slice(c * BL, (c + 1) * BL)
        in_maps.append(dict(
            x_k=x_k[sl],
            u=u_seq[sl].reshape(BL, M * CD),
            x_next=x_next_seq[sl].reshape(RT, SD),
            **weights,
        ))
    res = run_bass_kernel_spmd(nc, in_maps, list(range(NCORES)), trace=trace)
    z_pred = np.concatenate([r["z_pred"] for r in res.results], axis=0)
    x_pred = np.concatenate([r["x_pred"] for r in res.results], axis=0)
    z_target = np.concatenate([r["z_target"].reshape(BL, M, LD)
                               for r in res.results], axis=0)
    kernel.last_exec_time_ns = res.exec_time_ns
    kernel.last_results = res
    return (z_pred, x_pred, z_target)


# revision 7
# speedup vs baseline: 1.0211x; 1.0211x over previous
"""DeepKoopman linear-decoder kernel for 8 TRN2 NeuronCores.

Data-parallel over batch: each core gets B/8 = 256 rows; weights replicated.

Per core:
  - Encoder MLP (32->512->512->512->128, relu) in feature-major layout
    ([features, rows]) so layers chain without transposes; weights stationary.
  - All layout changes are REGULAR matmuls against an identity rhs
    (out = lhsT.T @ I) -- no transpose-mode instructions (those don't count
    as PE-busy for the HAM clock governor and throttle the array).
  - L3 for z_target is computed directly row-major (h2 chunks as lhsT,
    W3 natural as rhs), so z_target needs no back-transpose at all.
  - The 64-step recurrence z' = z@A + u@B is two accumulating matmuls per
    step into one PSUM bank (A on z^T; block-diagonal B on pre-transposed
    u), interleaved 2 steps per encoder tile so its serial chain hides.
  - z_pred row-major output AND the decoder x_pred come from ONE matmul
    per 128-row chunk: rhs = [I_128 | C_w] (N=160). Biases enter as K=1
    rank-1 matmuls (ones^T @ bias_row) only when nonzero.

Matmul dtype selectable: float32r (TF32-like, ~3e-4 rel err) default;
bfloat16 (~5e-3) or float32 (exact, 4x slower) also supported.
"""
import sys
sys.path.insert(0, "/opt/trn_rl_repo")

import numpy as np
from contextlib import ExitStack

import concourse.bacc as bacc
import concourse.tile as tile
from concourse.tile import TileContext
from concourse import mybir
from concourse.bass_utils import run_bass_kernel_spmd
from concourse.masks import make_identity

dt = mybir.dt
AF = mybir.ActivationFunctionType
ALU = mybir.AluOpType

B, M, SD, CD, LD, HW = 2048, 64, 32, 8, 128, 512
NCORES = 8
BL = B // NCORES          # 256 batch rows per core
RT = BL * M               # 16384 encoder rows per core
P = 128
NT = RT // 512            # 32 encoder tiles of 512 rows

_prog_cache = {}


def _build(mm_dt, use_bias):
    """use_bias: (b0, b1, b2, b3, cb) nonzero flags."""
    ub0, ub1, ub2, ub3, ubc = use_bias
    nc = bacc.Bacc()

    x_k_d = nc.declare_dram_parameter("x_k", [BL, SD], dt.float32, isOutput=False)
    u_d = nc.declare_dram_parameter("u", [BL, M * CD], dt.float32, isOutput=False)
    xn_d = nc.declare_dram_parameter("x_next", [RT, SD], dt.float32, isOutput=False)
    w0_d = nc.declare_dram_parameter("w0", [SD, HW], dt.float32, isOutput=False)
    b0_d = nc.declare_dram_parameter("b0", [HW], dt.float32, isOutput=False)
    w1_d = nc.declare_dram_parameter("w1", [HW, HW], dt.float32, isOutput=False)
    b1_d = nc.declare_dram_parameter("b1", [HW], dt.float32, isOutput=False)
    w2_d = nc.declare_dram_parameter("w2", [HW, HW], dt.float32, isOutput=False)
    b2_d = nc.declare_dram_parameter("b2", [HW], dt.float32, isOutput=False)
    w3_d = nc.declare_dram_parameter("w3", [HW, LD], dt.float32, isOutput=False)
    b3_d = nc.declare_dram_parameter("b3", [LD], dt.float32, isOutput=False)
    aw_d = nc.declare_dram_parameter("aw", [LD, LD], dt.float32, isOutput=False)
    bw_d = nc.declare_dram_parameter("bw", [CD, LD], dt.float32, isOutput=False)
    cw_d = nc.declare_dram_parameter("cw", [LD, SD], dt.float32, isOutput=False)
    cb_d = nc.declare_dram_parameter("cb", [SD], dt.float32, isOutput=False)

    zp_d = nc.declare_dram_parameter("z_pred", [BL, M, LD], dt.float32, isOutput=True)
    xp_d = nc.declare_dram_parameter("x_pred", [BL, M, SD], dt.float32, isOutput=True)
    zt_d = nc.declare_dram_parameter("z_target", [RT, LD], dt.float32, isOutput=True)

    xn_v = xn_d[:].rearrange("(l p) c -> p l c", p=P)        # [128, 128blk, 32]
    xk_v = x_k_d[:].rearrange("(l p) c -> p l c", p=P)       # [128, 2, 32]
    u_v = u_d[:].rearrange("(l p) c -> p l c", p=P)          # [128, 2, 512]
    zt_v = zt_d[:].rearrange("(g p) l -> p g l", p=P)        # [128, 128blk, 128]
    zp_v = zp_d[:].rearrange("(ch p) m l -> p ch m l", p=P)  # [128, 2, 64, 128]
    xp_v = xp_d[:].rearrange("(ch p) m c -> p ch m c", p=P)  # [128, 2, 64, 32]

    with TileContext(nc) as tc, ExitStack() as ctx:
        consts = ctx.enter_context(tc.tile_pool(name="consts", bufs=1))
        wst = ctx.enter_context(tc.tile_pool(name="wst", bufs=1))
        xin = ctx.enter_context(tc.tile_pool(name="xin", bufs=2))
        acts = ctx.enter_context(tc.tile_pool(name="acts", bufs=2))
        zts = ctx.enter_context(tc.tile_pool(name="zts", bufs=2))
        zpool = ctx.enter_context(tc.tile_pool(name="zpool", bufs=3))
        zps = ctx.enter_context(tc.tile_pool(name="zps", bufs=2))
        xps = ctx.enter_context(tc.tile_pool(name="xps", bufs=1))
        pe_ps = ctx.enter_context(tc.tile_pool(name="pe_ps", bufs=3, space="PSUM"))
        sm_ps = ctx.enter_context(tc.tile_pool(name="sm_ps", bufs=4, space="PSUM"))
        sc_ps = ctx.enter_context(tc.tile_pool(name="sc_ps", bufs=1, space="PSUM"))

        # ---- inputs first on the DMA queue (u + x_k) so the PE can start
        # transposing while the big weights stream in.
        u_in = wst.tile([P, 2, M * CD], dt.float32, tag="uin")
        nc.sync.dma_start(out=u_in, in_=u_v)
        xk_in = wst.tile([P, 2, SD], dt.float32, tag="xkin")
        nc.sync.dma_start(out=xk_in, in_=xk_v)

        # ---- identities
        ident32 = consts.tile([P, P], dt.float32)
        make_identity(nc, ident32)
        identr = consts.tile([P, P], mm_dt)
        nc.vector.tensor_copy(identr, ident32)

        # ---- weights: fp32 load -> mm_dt cast
        w0_st = wst.tile([SD, HW], dt.float32, tag="w0stage")
        nc.sync.dma_start(out=w0_st, in_=w0_d[:])
        w0r = consts.tile([SD, HW], mm_dt)
        nc.vector.tensor_copy(w0r, w0_st)
        w1r = consts.tile([P, 4, HW], mm_dt)
        w2r = consts.tile([P, 4, HW], mm_dt)
        for wd, wr in ((w1_d, w1r), (w2_d, w2r)):
            st = wst.tile([P, 4, HW], dt.float32, tag="wstage")
            nc.sync.dma_start(out=st, in_=wd[:].rearrange("(k p) o -> p k o", p=P))
            nc.vector.tensor_copy(wr, st)
        w3_st = wst.tile([P, 4, LD], dt.float32, tag="w3stage")
        nc.sync.dma_start(out=w3_st, in_=w3_d[:].rearrange("(k p) o -> p k o", p=P))
        w3r = consts.tile([P, 4, LD], mm_dt)
        nc.vector.tensor_copy(w3r, w3_st)
        aw_st = wst.tile([P, LD], dt.float32, tag="awstage")
        nc.sync.dma_start(out=aw_st, in_=aw_d[:])
        awr = consts.tile([P, LD], mm_dt)
        nc.vector.tensor_copy(awr, aw_st)
        # izc = [I_128 | C_w] used as rhs for the fused z_pred/x_pred step
        cw_st = wst.tile([P, SD], dt.float32, tag="cwstage")
        nc.sync.dma_start(out=cw_st, in_=cw_d[:])
        izc_st = wst.tile([P, P + SD], dt.float32, tag="izcstage")
        nc.vector.tensor_copy(izc_st[:, 0:P], ident32)
        nc.vector.tensor_copy(izc_st[:, P:P + SD], cw_st)
        izc = consts.tile([P, P + SD], mm_dt)
        nc.vector.tensor_copy(izc, izc_st)
        # B_w -> block-diagonal Bblk [128(16m x 8c), 16, 128]
        bw_st = wst.tile([CD, LD], dt.float32, tag="bwstage")
        nc.sync.dma_start(out=bw_st, in_=bw_d[:])
        bblk_st = wst.tile([P, 16, LD], dt.float32, tag="bblkstage")
        nc.vector.memset(bblk_st, 0.0)
        for mp in range(16):
            nc.sync.dma_start(out=bblk_st[8 * mp:8 * mp + 8, mp, :], in_=bw_st)
        bblk = consts.tile([P, 16, LD], mm_dt)
        nc.vector.tensor_copy(bblk, bblk_st)
        # biases
        b01 = consts.tile([P, 4], dt.float32, tag="b0t")
        nc.sync.dma_start(out=b01, in_=b0_d[:].rearrange("(k p) -> p k", p=P))
        b11 = consts.tile([P, 4], dt.float32, tag="b1t")
        nc.sync.dma_start(out=b11, in_=b1_d[:].rearrange("(k p) -> p k", p=P))
        b21 = consts.tile([P, 4], dt.float32, tag="b2t")
        nc.sync.dma_start(out=b21, in_=b2_d[:].rearrange("(k p) -> p k", p=P))
        b31 = consts.tile([P, 1], dt.float32, tag="b3t")
        nc.sync.dma_start(out=b31, in_=b3_d[:].rearrange("(p o) -> p o", o=1))
        bias_tiles = [b01, b11, b21]
        use_hid = [ub0, ub1, ub2]
        # rank-1 bias rows for row-major outputs (K=1 matmul operands)
        ones1 = None
        b3row = None
        cbrow = None
        if ub3 or ubc:
            ones_st = wst.tile([1, P], dt.float32, tag="onesstage")
            nc.vector.memset(ones_st, 1.0)
            ones1 = consts.tile([1, P], mm_dt)
            nc.vector.tensor_copy(ones1, ones_st)
        if ub3:
            b3r_st = wst.tile([1, LD], dt.float32, tag="b3rstage")
            nc.sync.dma_start(out=b3r_st, in_=b3_d[:].rearrange("(o l) -> o l", o=1))
            b3row = consts.tile([1, LD], mm_dt)
            nc.vector.tensor_copy(b3row, b3r_st)
        if ubc:
            cbr_st = wst.tile([1, P + SD], dt.float32, tag="cbrstage")
            nc.vector.memset(cbr_st, 0.0)
            nc.sync.dma_start(out=cbr_st[:, P:P + SD],
                              in_=cb_d[:].rearrange("(o c) -> o c", o=1))
            cbrow = consts.tile([1, P + SD], mm_dt)
            nc.vector.tensor_copy(cbrow, cbr_st)

        # ---- u prep: transpose [128b, 128(16m x 8c)] blocks via regular
        # matmul against identity -> uT[j][128(m,c), 2ch, 128b] in mm_dt
        ur = wst.tile([P, 2, M * CD], mm_dt, tag="ur")
        nc.vector.tensor_copy(ur, u_in)
        uT = [consts.tile([P, 2, P], mm_dt, tag=f"uT{j}", name=f"uT{j}")
              for j in range(4)]
        for ch in range(2):
            for j in range(4):
                pst = sm_ps.tile([P, P], dt.float32, tag="sm")
                nc.tensor.matmul(pst, ur[:, ch, j * P:(j + 1) * P], identr,
                                 start=True, stop=True)
                nc.vector.tensor_copy(uT[j][:, ch, :], pst)

        # ---- encoder hidden layers (feature-major), n rows at a time
        def encode_hidden(xT, n):
            h_prev = None
            for li, wr in enumerate((w0r, w1r, w2r)):
                h = acts.tile([P, 4, 512], mm_dt, tag=f"h{li}")
                for j in range(4):
                    ps = pe_ps.tile([P, 512], dt.float32, tag="ps")
                    if li == 0:
                        nc.tensor.matmul(ps[:, :n], w0r[:, j * P:(j + 1) * P],
                                         xT, start=True, stop=True)
                    else:
                        for kk in range(4):
                            nc.tensor.matmul(
                                ps[:, :n], wr[:, kk, j * P:(j + 1) * P],
                                h_prev[:, kk, :n],
                                start=(kk == 0), stop=(kk == 3))
                    if use_hid[li]:
                        if j % 2 == 0:
                            nc.scalar.activation(h[:, j, :n], ps[:, :n], AF.Relu,
                                                 bias=bias_tiles[li][:, j:j + 1],
                                                 scale=1.0)
                        else:
                            nc.vector.tensor_scalar(
                                out=h[:, j, :n], in0=ps[:, :n],
                                scalar1=bias_tiles[li][:, j:j + 1], scalar2=0.0,
                                op0=ALU.add, op1=ALU.max)
                    else:
                        if j % 2 == 0:
                            nc.scalar.activation(h[:, j, :n], ps[:, :n],
                                                 AF.Relu, scale=1.0)
                        else:
                            nc.vector.tensor_scalar(
                                out=h[:, j, :n], in0=ps[:, :n],
                                scalar1=0.0, scalar2=None, op0=ALU.max)
                h_prev = h
            return h_prev

        # ---- x_k encode -> z_cur^T [128, 2, 128] (feature-major L3)
        xkr = wst.tile([P, 2, SD], mm_dt, tag="xkr")
        nc.vector.tensor_copy(xkr, xk_in)
        xkT = acts.tile([SD, 2, P], mm_dt, tag="xT")
        for ch in range(2):
            pst = sm_ps.tile([SD, P], dt.float32, tag="sm")
            nc.tensor.matmul(pst, xkr[:, ch, :], identr, start=True, stop=True)
            nc.vector.tensor_copy(xkT[:, ch, :], pst)
        hk2 = encode_hidden(xkT, BL)
        psk = pe_ps.tile([P, 512], dt.float32, tag="ps")
        for kk in range(4):
            nc.tensor.matmul(psk[:, :BL], w3r[:, kk, :], hk2[:, kk, :BL],
                             start=(kk == 0), stop=(kk == 3))
        z0 = zpool.tile([P, 2, P], mm_dt, tag="zcur")
        if ub3:
            nc.vector.tensor_scalar(out=z0[:, 0, :], in0=psk[:, 0:P],
                                    scalar1=b31, scalar2=None, op0=ALU.add)
            nc.vector.tensor_scalar(out=z0[:, 1, :], in0=psk[:, P:2 * P],
                                    scalar1=b31, scalar2=None, op0=ALU.add)
        else:
            nc.vector.tensor_copy(z0, psk[:, :BL].rearrange("p (c b) -> p c b", c=2))
        z_state = [z0]

        # ---- scan
        scan_ctx = {"xp": [xps.tile([P, M, SD], dt.float32, tag=f"xps{c}",
                                    name=f"xps{c}") for c in range(2)]}

        def scan_step(m):
            q, mi = divmod(m, 16)
            if mi == 0:
                scan_ctx["zq"] = [zps.tile([P, 16, LD], dt.float32,
                                           tag=f"zps{c}", name=f"zps{c}")
                                  for c in range(2)]
            psA = sc_ps.tile([P, 2 * P], dt.float32, tag="sps")
            nc.tensor.matmul(psA, awr, z_state[0], start=True, stop=False)
            nc.tensor.matmul(psA, bblk[:, m % 16, :], uT[q], start=False, stop=True)
            znext = zpool.tile([P, 2, P], mm_dt, tag="zcur")
            nc.vector.tensor_copy(znext, psA.rearrange("p (c b) -> p c b", c=2))
            z_state[0] = znext
            # fused row-major z_pred + decoder x_pred: out = znext_ch.T @ [I|C]
            for ch in range(2):
                pzx = sm_ps.tile([P, P + SD], dt.float32, tag="sm")
                nc.tensor.matmul(pzx, znext[:, ch, :], izc,
                                 start=True, stop=not ubc)
                if ubc:
                    nc.tensor.matmul(pzx, ones1, cbrow, start=False, stop=True)
                nc.scalar.activation(scan_ctx["zq"][ch][:, mi, :], pzx[:, 0:P],
                                     AF.Copy)
                nc.scalar.activation(scan_ctx["xp"][ch][:, m, :], pzx[:, P:P + SD],
                                     AF.Copy)
            if mi == 15:
                for ch in range(2):
                    nc.sync.dma_start(out=zp_v[:, ch, q * 16:(q + 1) * 16, :],
                                      in_=scan_ctx["zq"][ch])

        # ---- main loop: 32 encoder tiles, 2 scan steps interleaved per tile
        for t in range(NT):
            if t % 8 == 0:
                xt_in = xin.tile([P, 32, SD], dt.float32, tag="xin")
                nc.sync.dma_start(out=xt_in,
                                  in_=xn_v[:, (t // 8) * 32:(t // 8) * 32 + 32, :])
            if t % 2 == 0:
                zt_stage = zts.tile([P, 8, LD], dt.float32, tag="zts")
            # input rows -> feature-major via identity-matmuls
            xr = acts.tile([P, 4 * SD], mm_dt, tag="xr")
            nc.vector.tensor_copy(
                xr, xt_in[:, (t % 8) * 4:(t % 8) * 4 + 4, :].rearrange(
                    "p a c -> p (a c)"))
            xT = acts.tile([SD, 4, P], mm_dt, tag="xT")
            for q in range(4):
                pst = sm_ps.tile([SD, P], dt.float32, tag="sm")
                nc.tensor.matmul(pst, xr[:, q * SD:(q + 1) * SD], identr,
                                 start=True, stop=True)
                nc.vector.tensor_copy(xT[:, q, :], pst)
            h2 = encode_hidden(xT.rearrange("p q b -> p (q b)"), 512)
            # L3 directly row-major: z[rc-chunk] = h2_chunk.T @ W3 (+ b3)
            for rc in range(4):
                psz = sm_ps.tile([P, LD], dt.float32, tag="sm")
                for kk in range(4):
                    nc.tensor.matmul(psz, h2[:, kk, rc * P:(rc + 1) * P],
                                     w3r[:, kk, :], start=(kk == 0),
                                     stop=(kk == 3 and not ub3))
                if ub3:
                    nc.tensor.matmul(psz, ones1, b3row, start=False, stop=True)
                gg = (t % 2) * 4 + rc
                if rc % 2 == 0:
                    nc.scalar.activation(zt_stage[:, gg, :], psz, AF.Copy)
                else:
                    nc.vector.tensor_copy(zt_stage[:, gg, :], psz)
            if t % 2 == 1:
                nc.sync.dma_start(out=zt_v[:, (t - 1) * 4:(t - 1) * 4 + 8, :],
                                  in_=zt_stage)
            scan_step(2 * t)
            scan_step(2 * t + 1)

        for ch in range(2):
            nc.sync.dma_start(out=xp_v[:, ch, :, :], in_=scan_ctx["xp"][ch])

    nc.compile()
    return nc


def _get_prog(mm_dt_name, use_bias):
    key = (mm_dt_name, use_bias)
    if key not in _prog_cache:
        _prog_cache[key] = _build(getattr(dt, mm_dt_name), use_bias)
    return _prog_cache[key]


def kernel(x_k, u_seq, x_next_seq,
           enc_w0, enc_b0, enc_w1, enc_b1, enc_w2, enc_b2, enc_w3, enc_b3,
           A_w, B_w, C_w, C_b, mm_dt_name="float32r", trace=False):
    x_k = np.ascontiguousarray(x_k, dtype=np.float32)
    u_seq = np.ascontiguousarray(u_seq, dtype=np.float32)
    x_next_seq = np.ascontiguousarray(x_next_seq, dtype=np.float32)
    weights = dict(
        w0=np.ascontiguousarray(enc_w0, np.float32),
        b0=np.ascontiguousarray(enc_b0, np.float32),
        w1=np.ascontiguousarray(enc_w1, np.float32),
        b1=np.ascontiguousarray(enc_b1, np.float32),
        w2=np.ascontiguousarray(enc_w2, np.float32),
        b2=np.ascontiguousarray(enc_b2, np.float32),
        w3=np.ascontiguousarray(enc_w3, np.float32),
        b3=np.ascontiguousarray(enc_b3, np.float32),
        aw=np.ascontiguousarray(A_w, np.float32),
        bw=np.ascontiguousarray(B_w, np.float32),
        cw=np.ascontiguousarray(C_w, np.float32),
        cb=np.ascontiguousarray(C_b, np.float32),
    )
    use_bias = tuple(bool(np.any(weights[k])) for k in ("b0", "b1", "b2", "b3", "cb"))
    nc = _get_prog(mm_dt_name, use_bias)
    in_maps = []
    for c in range(NCORES):
        sl = slice(c * BL, (c + 1) * BL)
        in_maps.append(dict(
            x_k=x_k[sl],
            u=u_seq[sl].reshape(BL, M * CD),
            x_next=x_next_seq[sl].reshape(RT, SD),
            **weights,
        ))
    res = run_bass_kernel_spmd(nc, in_maps, list(range(NCORES)), trace=trace)
    z_pred = np.concatenate([r["z_pred"] for r in res.results], axis=0)
    x_pred = np.concatenate([r["x_pred"] for r in res.results], axis=0)
    z_target = np.concatenate([r["z_target"].reshape(BL, M, LD)
                               for r in res.results], axis=0)
    kernel.last_exec_time_ns = res.exec_time_ns
    kernel.last_results = res
    return (z_pred, x_pred, z_target)


# revision 10
# speedup vs baseline: 1.5555x; 1.5234x over previous
"""DeepKoopman linear-decoder kernel for 8 TRN2 NeuronCores.

Data-parallel over batch: each core gets B/8 = 256 rows; weights replicated.

Per core:
  - Encoder MLP (32->512->512->512->128, relu) in feature-major layout
    ([features, rows]) so layers chain without transposes; weights stationary.
  - All layout changes are REGULAR matmuls against an identity rhs
    (out = lhsT.T @ I) -- no transpose-mode instructions (those don't count
    as PE-busy for the HAM clock governor and throttle the array).
  - L3 for z_target is computed directly row-major (h2 chunks as lhsT,
    W3 natural as rhs), so z_target needs no back-transpose at all.
  - The 64-step recurrence z' = z@A + u@B is two accumulating matmuls per
    step into one PSUM bank (A on z^T; block-diagonal B on pre-transposed
    u), interleaved 2 steps per encoder tile so its serial chain hides.
  - z_pred row-major output AND the decoder x_pred come from ONE matmul
    per 128-row chunk: rhs = [I_128 | C_w] (N=160). Biases enter as K=1
    rank-1 matmuls (ones^T @ bias_row) only when nonzero.

Matmul dtype selectable: float32r (TF32-like, ~3e-4 rel err) default;
bfloat16 (~5e-3) or float32 (exact, 4x slower) also supported.
"""
import sys
sys.path.insert(0, "/opt/trn_rl_repo")

import numpy as np
from contextlib import ExitStack

import concourse.bacc as bacc
import concourse.tile as tile
from concourse.tile import TileContext
from concourse import mybir
from concourse.bass_utils import run_bass_kernel_spmd
from concourse.masks import make_identity

dt = mybir.dt
AF = mybir.ActivationFunctionType
ALU = mybir.AluOpType

B, M, SD, CD, LD, HW = 2048, 64, 32, 8, 128, 512
NCORES = 8
BL = B // NCORES          # 256 batch rows per core
RT = BL * M               # 16384 encoder rows per core
P = 128
NT = RT // 512            # 32 encoder tiles of 512 rows

_prog_cache = {}


def _build(mm_dt, use_bias):
    """use_bias: (b0, b1, b2, b3, cb) nonzero flags."""
    ub0, ub1, ub2, ub3, ubc = use_bias
    nc = bacc.Bacc()

    x_k_d = nc.declare_dram_parameter("x_k", [BL, SD], dt.float32, isOutput=False)
    u_d = nc.declare_dram_parameter("u", [BL, M * CD], dt.float32, isOutput=False)
    xn_d = nc.declare_dram_parameter("x_next", [RT, SD], dt.float32, isOutput=False)
    w0_d = nc.declare_dram_parameter("w0", [SD, HW], dt.float32, isOutput=False)
    b0_d = nc.declare_dram_parameter("b0", [HW], dt.float32, isOutput=False)
    w1_d = nc.declare_dram_parameter("w1", [HW, HW], dt.float32, isOutput=False)
    b1_d = nc.declare_dram_parameter("b1", [HW], dt.float32, isOutput=False)
    w2_d = nc.declare_dram_parameter("w2", [HW, HW], dt.float32, isOutput=False)
    b2_d = nc.declare_dram_parameter("b2", [HW], dt.float32, isOutput=False)
    w3_d = nc.declare_dram_parameter("w3", [HW, LD], dt.float32, isOutput=False)
    b3_d = nc.declare_dram_parameter("b3", [LD], dt.float32, isOutput=False)
    aw_d = nc.declare_dram_parameter("aw", [LD, LD], dt.float32, isOutput=False)
    bw_d = nc.declare_dram_parameter("bw", [CD, LD], dt.float32, isOutput=False)
    cw_d = nc.declare_dram_parameter("cw", [LD, SD], dt.float32, isOutput=False)
    cb_d = nc.declare_dram_parameter("cb", [SD], dt.float32, isOutput=False)

    zp_d = nc.declare_dram_parameter("z_pred", [BL, M, LD], dt.float32, isOutput=True)
    xp_d = nc.declare_dram_parameter("x_pred", [BL, M, SD], dt.float32, isOutput=True)
    zt_d = nc.declare_dram_parameter("z_target", [RT, LD], dt.float32, isOutput=True)

    xn_v = xn_d[:].rearrange("(l p) c -> p l c", p=P)        # [128, 128blk, 32]
    xk_v = x_k_d[:].rearrange("(l p) c -> p l c", p=P)       # [128, 2, 32]
    u_v = u_d[:].rearrange("(l p) c -> p l c", p=P)          # [128, 2, 512]
    zt_v = zt_d[:].rearrange("(g p) l -> p g l", p=P)        # [128, 128blk, 128]
    zp_v = zp_d[:].rearrange("(ch p) m l -> p ch m l", p=P)  # [128, 2, 64, 128]
    xp_v = xp_d[:].rearrange("(ch p) m c -> p ch m c", p=P)  # [128, 2, 64, 32]

    with TileContext(nc) as tc, ExitStack() as ctx:
        consts = ctx.enter_context(tc.tile_pool(name="consts", bufs=1))
        wst = ctx.enter_context(tc.tile_pool(name="wst", bufs=1))
        xin = ctx.enter_context(tc.tile_pool(name="xin", bufs=2))
        acts = ctx.enter_context(tc.tile_pool(name="acts", bufs=2))
        zts = ctx.enter_context(tc.tile_pool(name="zts", bufs=2))
        zpool = ctx.enter_context(tc.tile_pool(name="zpool", bufs=3))
        zps = ctx.enter_context(tc.tile_pool(name="zps", bufs=2))
        xps = ctx.enter_context(tc.tile_pool(name="xps", bufs=1))
        pe_ps = ctx.enter_context(tc.tile_pool(name="pe_ps", bufs=3, space="PSUM"))
        sm_ps = ctx.enter_context(tc.tile_pool(name="sm_ps", bufs=4, space="PSUM"))
        sc_ps = ctx.enter_context(tc.tile_pool(name="sc_ps", bufs=1, space="PSUM"))

        # ---- inputs first on the DMA queue (u + x_k) so the PE can start
        # transposing while the big weights stream in.
        u_in = wst.tile([P, 2, M * CD], dt.float32, tag="uin")
        nc.sync.dma_start(out=u_in, in_=u_v)
        xk_in = wst.tile([P, 2, SD], dt.float32, tag="xkin")
        nc.sync.dma_start(out=xk_in, in_=xk_v)

        # ---- identities
        ident32 = consts.tile([P, P], dt.float32)
        make_identity(nc, ident32)
        identr = consts.tile([P, P], mm_dt)
        nc.vector.tensor_copy(identr, ident32)

        # ---- weights: fp32 load -> mm_dt cast
        w0_st = wst.tile([SD, HW], dt.float32, tag="w0stage")
        nc.sync.dma_start(out=w0_st, in_=w0_d[:])
        w0r = consts.tile([SD, HW], mm_dt)
        nc.vector.tensor_copy(w0r, w0_st)
        w1r = consts.tile([P, 4, HW], mm_dt)
        w2r = consts.tile([P, 4, HW], mm_dt)
        for wd, wr in ((w1_d, w1r), (w2_d, w2r)):
            st = wst.tile([P, 4, HW], dt.float32, tag="wstage")
            nc.sync.dma_start(out=st, in_=wd[:].rearrange("(k p) o -> p k o", p=P))
            nc.vector.tensor_copy(wr, st)
        w3_st = wst.tile([P, 4, LD], dt.float32, tag="w3stage")
        nc.sync.dma_start(out=w3_st, in_=w3_d[:].rearrange("(k p) o -> p k o", p=P))
        w3r = consts.tile([P, 4, LD], mm_dt)
        nc.vector.tensor_copy(w3r, w3_st)
        aw_st = wst.tile([P, LD], dt.float32, tag="awstage")
        nc.sync.dma_start(out=aw_st, in_=aw_d[:])
        awr = consts.tile([P, LD], mm_dt)
        nc.vector.tensor_copy(awr, aw_st)
        # izc = [I_128 | C_w] used as rhs for the fused z_pred/x_pred step
        cw_st = wst.tile([P, SD], dt.float32, tag="cwstage")
        nc.sync.dma_start(out=cw_st, in_=cw_d[:])
        izc_st = wst.tile([P, P + SD], dt.float32, tag="izcstage")
        nc.vector.tensor_copy(izc_st[:, 0:P], ident32)
        nc.vector.tensor_copy(izc_st[:, P:P + SD], cw_st)
        izc = consts.tile([P, P + SD], mm_dt)
        nc.vector.tensor_copy(izc, izc_st)
        # B_w -> block-diagonal Bblk [128(16m x 8c), 16, 128]
        bw_st = wst.tile([CD, LD], dt.float32, tag="bwstage")
        nc.sync.dma_start(out=bw_st, in_=bw_d[:])
        bblk_st = wst.tile([P, 16, LD], dt.float32, tag="bblkstage")
        nc.vector.memset(bblk_st, 0.0)
        for mp in range(16):
            nc.sync.dma_start(out=bblk_st[8 * mp:8 * mp + 8, mp, :], in_=bw_st)
        bblk = consts.tile([P, 16, LD], mm_dt)
        nc.vector.tensor_copy(bblk, bblk_st)
        # biases
        b01 = consts.tile([P, 4], dt.float32, tag="b0t")
        nc.sync.dma_start(out=b01, in_=b0_d[:].rearrange("(k p) -> p k", p=P))
        b11 = consts.tile([P, 4], dt.float32, tag="b1t")
        nc.sync.dma_start(out=b11, in_=b1_d[:].rearrange("(k p) -> p k", p=P))
        b21 = consts.tile([P, 4], dt.float32, tag="b2t")
        nc.sync.dma_start(out=b21, in_=b2_d[:].rearrange("(k p) -> p k", p=P))
        b31 = consts.tile([P, 1], dt.float32, tag="b3t")
        nc.sync.dma_start(out=b31, in_=b3_d[:].rearrange("(p o) -> p o", o=1))
        bias_tiles = [b01, b11, b21]
        use_hid = [ub0, ub1, ub2]
        # rank-1 bias rows for row-major outputs (K=1 matmul operands)
        ones1 = None
        b3row = None
        cbrow = None
        if ub3 or ubc:
            ones_st = wst.tile([1, P], dt.float32, tag="onesstage")
            nc.vector.memset(ones_st, 1.0)
            ones1 = consts.tile([1, P], mm_dt)
            nc.vector.tensor_copy(ones1, ones_st)
        if ub3:
            b3r_st = wst.tile([1, LD], dt.float32, tag="b3rstage")
            nc.sync.dma_start(out=b3r_st, in_=b3_d[:].rearrange("(o l) -> o l", o=1))
            b3row = consts.tile([1, LD], mm_dt)
            nc.vector.tensor_copy(b3row, b3r_st)
        if ubc:
            cbr_st = wst.tile([1, P + SD], dt.float32, tag="cbrstage")
            nc.vector.memset(cbr_st, 0.0)
            nc.sync.dma_start(out=cbr_st[:, P:P + SD],
                              in_=cb_d[:].rearrange("(o c) -> o c", o=1))
            cbrow = consts.tile([1, P + SD], mm_dt)
            nc.vector.tensor_copy(cbrow, cbr_st)

        # ---- u prep: transpose [128b, 128(16m x 8c)] blocks via regular
        # matmul against identity -> uT[j][128(m,c), 2ch, 128b] in mm_dt
        ur = wst.tile([P, 2, M * CD], mm_dt, tag="ur")
        nc.vector.tensor_copy(ur, u_in)
        uT = [consts.tile([P, 2, P], mm_dt, tag=f"uT{j}", name=f"uT{j}")
              for j in range(4)]
        for ch in range(2):
            for j in range(4):
                pst = sm_ps.tile([P, P], dt.float32, tag="sm")
                nc.tensor.matmul(pst, ur[:, ch, j * P:(j + 1) * P], identr,
                                 start=True, stop=True)
                nc.vector.tensor_copy(uT[j][:, ch, :], pst)

        # ---- relu+bias PSUM->SBUF evict, alternating ACT/DVE
        def _relu_copy(out_ap, ps_ap, li, j):
            if use_hid[li]:
                if j % 2 == 0:
                    nc.scalar.activation(out_ap, ps_ap, AF.Relu,
                                         bias=bias_tiles[li][:, j:j + 1],
                                         scale=1.0)
                else:
                    nc.vector.tensor_scalar(
                        out=out_ap, in0=ps_ap,
                        scalar1=bias_tiles[li][:, j:j + 1], scalar2=0.0,
                        op0=ALU.add, op1=ALU.max)
            else:
                if j % 2 == 0:
                    nc.scalar.activation(out_ap, ps_ap, AF.Relu, scale=1.0)
                else:
                    nc.vector.tensor_scalar(out=out_ap, in0=ps_ap,
                                            scalar1=0.0, scalar2=None,
                                            op0=ALU.max)

        # ---- encoder hidden layers (feature-major), n rows at a time
        def encode_hidden(xT, n):
            h_prev = None
            for li, wr in enumerate((w0r, w1r, w2r)):
                h = acts.tile([P, 4, 512], mm_dt, tag=f"h{li}")
                for j in range(4):
                    ps = pe_ps.tile([P, 512], dt.float32, tag="ps")
                    if li == 0:
                        nc.tensor.matmul(ps[:, :n], w0r[:, j * P:(j + 1) * P],
                                         xT, start=True, stop=True)
                    else:
                        for kk in range(4):
                            nc.tensor.matmul(
                                ps[:, :n], wr[:, kk, j * P:(j + 1) * P],
                                h_prev[:, kk, :n],
                                start=(kk == 0), stop=(kk == 3))
                    _relu_copy(h[:, j, :n], ps[:, :n], li, j)
                h_prev = h
            return h_prev

        # ---- x_k encode -> z_cur^T [128, 2, 128] (feature-major L3)
        xkr = wst.tile([P, 2, SD], mm_dt, tag="xkr")
        nc.vector.tensor_copy(xkr, xk_in)
        xkT = acts.tile([SD, 2, P], mm_dt, tag="xT")
        for ch in range(2):
            pst = sm_ps.tile([SD, P], dt.float32, tag="sm")
            nc.tensor.matmul(pst, xkr[:, ch, :], identr, start=True, stop=True)
            nc.vector.tensor_copy(xkT[:, ch, :], pst)
        hk2 = encode_hidden(xkT, BL)
        psk = pe_ps.tile([P, 512], dt.float32, tag="ps")
        for kk in range(4):
            nc.tensor.matmul(psk[:, :BL], w3r[:, kk, :], hk2[:, kk, :BL],
                             start=(kk == 0), stop=(kk == 3))
        z0 = zpool.tile([P, 2, P], mm_dt, tag="zcur")
        if ub3:
            nc.vector.tensor_scalar(out=z0[:, 0, :], in0=psk[:, 0:P],
                                    scalar1=b31, scalar2=None, op0=ALU.add)
            nc.vector.tensor_scalar(out=z0[:, 1, :], in0=psk[:, P:2 * P],
                                    scalar1=b31, scalar2=None, op0=ALU.add)
        else:
            nc.vector.tensor_copy(z0, psk[:, :BL].rearrange("p (c b) -> p c b", c=2))
        z_state = [z0]

        # ---- scan
        scan_ctx = {"xp": [xps.tile([P, M, SD], dt.float32, tag=f"xps{c}",
                                    name=f"xps{c}") for c in range(2)]}

        def scan_step(m):
            q, mi = divmod(m, 16)
            if mi == 0:
                scan_ctx["zq"] = [zps.tile([P, 16, LD], dt.float32,
                                           tag=f"zps{c}", name=f"zps{c}")
                                  for c in range(2)]
            psA = sc_ps.tile([P, 2 * P], dt.float32, tag="sps")
            nc.tensor.matmul(psA, awr, z_state[0], start=True, stop=False)
            nc.tensor.matmul(psA, bblk[:, m % 16, :], uT[q], start=False, stop=True)
            znext = zpool.tile([P, 2, P], mm_dt, tag="zcur")
            nc.vector.tensor_copy(znext, psA.rearrange("p (c b) -> p c b", c=2))
            z_state[0] = znext
            # fused row-major z_pred + decoder x_pred: out = znext_ch.T @ [I|C]
            for ch in range(2):
                pzx = sm_ps.tile([P, P + SD], dt.float32, tag="sm")
                nc.tensor.matmul(pzx, znext[:, ch, :], izc,
                                 start=True, stop=not ubc)
                if ubc:
                    nc.tensor.matmul(pzx, ones1, cbrow, start=False, stop=True)
                nc.scalar.activation(scan_ctx["zq"][ch][:, mi, :], pzx[:, 0:P],
                                     AF.Copy)
                nc.scalar.activation(scan_ctx["xp"][ch][:, m, :], pzx[:, P:P + SD],
                                     AF.Copy)
            if mi == 15:
                for ch in range(2):
                    nc.sync.dma_start(out=zp_v[:, ch, q * 16:(q + 1) * 16, :],
                                      in_=scan_ctx["zq"][ch])

        # ---- main loop: 32 encoder tiles, software-pipelined.
        # xT for tile t+1 is computed mid-tile-t; the two scan steps are
        # split across the tile so the PE never waits on the DVE chain.
        xin_tiles = {}

        def load_block(b):
            xt = xin.tile([P, 32, SD], dt.float32, tag="xin")
            nc.sync.dma_start(out=xt, in_=xn_v[:, b * 32:b * 32 + 32, :])
            xin_tiles[b] = xt

        def make_xT(t):
            xt_in = xin_tiles[t // 8]
            xr = acts.tile([P, 4 * SD], mm_dt, tag="xr")
            nc.vector.tensor_copy(
                xr, xt_in[:, (t % 8) * 4:(t % 8) * 4 + 4, :].rearrange(
                    "p a c -> p (a c)"))
            xT = acts.tile([SD, 4, P], mm_dt, tag="xT")
            for q in range(4):
                pst = sm_ps.tile([SD, P], dt.float32, tag="sm")
                nc.tensor.matmul(pst, xr[:, q * SD:(q + 1) * SD], identr,
                                 start=True, stop=True)
                nc.vector.tensor_copy(xT[:, q, :], pst)
            return xT

        load_block(0)
        xT_next = make_xT(0)
        for t in range(NT):
            if t % 8 == 0 and t // 8 + 1 < (NT + 7) // 8:
                load_block(t // 8 + 1)
            if t % 2 == 0:
                zt_stage = zts.tile([P, 8, LD], dt.float32, tag="zts")
            xT = xT_next
            # L0
            xTflat = xT.rearrange("p q b -> p (q b)")
            h0 = acts.tile([P, 4, 512], mm_dt, tag="h0")
            for j in range(4):
                ps = pe_ps.tile([P, 512], dt.float32, tag="ps")
                nc.tensor.matmul(ps, w0r[:, j * P:(j + 1) * P], xTflat,
                                 start=True, stop=True)
                _relu_copy(h0[:, j, :], ps, 0, j)
            scan_step(2 * t)
            # L1
            h1 = acts.tile([P, 4, 512], mm_dt, tag="h1")
            for j in range(4):
                ps = pe_ps.tile([P, 512], dt.float32, tag="ps")
                for kk in range(4):
                    nc.tensor.matmul(ps, w1r[:, kk, j * P:(j + 1) * P],
                                     h0[:, kk, :], start=(kk == 0), stop=(kk == 3))
                _relu_copy(h1[:, j, :], ps, 1, j)
            # prefetch next tile's transposed input while PE is saturated
            if t + 1 < NT:
                xT_next = make_xT(t + 1)
            # L2
            h2 = acts.tile([P, 4, 512], mm_dt, tag="h2")
            for j in range(4):
                ps = pe_ps.tile([P, 512], dt.float32, tag="ps")
                for kk in range(4):
                    nc.tensor.matmul(ps, w2r[:, kk, j * P:(j + 1) * P],
                                     h1[:, kk, :], start=(kk == 0), stop=(kk == 3))
                _relu_copy(h2[:, j, :], ps, 2, j)
            scan_step(2 * t + 1)
            # L3 directly row-major: z[rc-chunk] = h2_chunk.T @ W3 (+ b3)
            for rc in range(4):
                psz = sm_ps.tile([P, LD], dt.float32, tag="sm")
                for kk in range(4):
                    nc.tensor.matmul(psz, h2[:, kk, rc * P:(rc + 1) * P],
                                     w3r[:, kk, :], start=(kk == 0),
                                     stop=(kk == 3 and not ub3))
                if ub3:
                    nc.tensor.matmul(psz, ones1, b3row, start=False, stop=True)
                gg = (t % 2) * 4 + rc
                if rc % 2 == 0:
                    nc.scalar.activation(zt_stage[:, gg, :], psz, AF.Copy)
                else:
                    nc.vector.tensor_copy(zt_stage[:, gg, :], psz)
            if t % 2 == 1:
                nc.sync.dma_start(out=zt_v[:, (t - 1) * 4:(t - 1) * 4 + 8, :],
                                  in_=zt_stage)

        for ch in range(2):
            nc.sync.dma_start(out=xp_v[:, ch, :, :], in_=scan_ctx["xp"][ch])

    nc.compile()
    return nc


def _get_prog(mm_dt_name, use_bias):
    key = (mm_dt_name, use_bias)
    if key not in _prog_cache:
        _prog_cache[key] = _build(getattr(dt, mm_dt_name), use_bias)
    return _prog_cache[key]


def kernel(x_k, u_seq, x_next_seq,
           enc_w0, enc_b0, enc_w1, enc_b1, enc_w2, enc_b2, enc_w3, enc_b3,
           A_w, B_w, C_w, C_b, mm_dt_name="float32r", trace=False):
    x_k = np.ascontiguousarray(x_k, dtype=np.float32)
    u_seq = np.ascontiguousarray(u_seq, dtype=np.float32)
    x_next_seq = np.ascontiguousarray(x_next_seq, dtype=np.float32)
    weights = dict(
        w0=np.ascontiguousarray(enc_w0, np.float32),
        b0=np.ascontiguousarray(enc_b0, np.float32),
        w1=np.ascontiguousarray(enc_w1, np.float32),
        b1=np.ascontiguousarray(enc_b1, np.float32),
        w2=np.ascontiguousarray(enc_w2, np.float32),
        b2=np.ascontiguousarray(enc_b2, np.float32),
        w3=np.ascontiguousarray(enc_w3, np.float32),
        b3=np.ascontiguousarray(enc_b3, np.float32),
        aw=np.ascontiguousarray(A_w, np.float32),
        bw=np.ascontiguousarray(B_w, np.float32),
        cw=np.ascontiguousarray(C_w, np.float32),
        cb=np.ascontiguousarray(C_b, np.float32),
    )
    use_bias = tuple(bool(np.any(weights[k])) for k in ("b0", "b1", "b2", "b3", "cb"))
    nc = _get_prog(mm_dt_name, use_bias)
    in_maps = []
    for c in range(NCORES):
        sl = slice(c * BL, (c + 1) * BL)
        in_maps.append(dict(
            x_k=x_k[sl],
            u=u_seq[sl].reshape(BL, M * CD),
            x_next=x_next_seq[sl].reshape(RT, SD),
            **weights,
        ))
    res = run_bass_kernel_spmd(nc, in_maps, list(range(NCORES)), trace=trace)
    z_pred = np.concatenate([r["z_pred"] for r in res.results], axis=0)
    x_pred = np.concatenate([r["x_pred"] for r in res.results], axis=0)
    z_target = np.concatenate([r["z_target"].reshape(BL, M, LD)
                               for r in res.results], axis=0)
    kernel.last_exec_time_ns = res.exec_time_ns
    kernel.last_results = res
    return (z_pred, x_pred, z_target)
